# revision 15
# baseline (speedup 1.0000x reference)
"""Trainium2 Bass kernel for the fused GNN message-passing block.

Reference computation (per batch b):
    h = silu(x @ W1 + b1) @ W2 + b2                       # [K, C]
    out[q, d, c] = sum_k mask[q,k] * ev[q,k,d] * ef[q,k,c] * h[k,c]

Sharding: data-parallel over (b, q-half) -> 8 cores, each core handles
one b (of 4) and 64 of the 128 q values.  All large tensors carry the
leading b dim; the tiny MLP weights are replicated.

v3 layout/precision strategy:
  - ef is cast to bf16 on the host: halves the dominant HBM stream
    (8.4 -> 4.2 MB per core; one HWDGE queue sustains ~420 GB/s),
    doubles DVE throughput of the big ef*h multiply (2x 16-bit mode),
    and quadruples PE matmul column rate vs fp32.
  - The MLP runs in fp32; h is cast to bf16 and replicated x8 along q
    (log2 doubling copies) so the main multiply has unit-stride 16-bit
    operands.  w = (mask*ev)^T is built in fp32 (strided bf16 DVE
    writes corrupt neighboring 16-bit lanes) and cast densely on
    gpsimd.
  - HWDGE descriptor generation costs ~0.7 us of engine-queue time per
    dma_start, so the five MLP weight tensors ship as ONE packed param
    and ev/mask as another; ef streams as 8 x 512 KiB on the sync
    queue, in consumption order, behind the weight pack.
  - A burst of dep-free bf16 matmuls bridges the PE idle window while
    the weights land, so the HAM activity monitor reaches 8/8 before
    the MLP and main loop (cold PE runs at 1.2 GHz, warm at 2.4).
  - Outputs accumulate in SBUF and leave as ONE DMA at the end; the
    host unscrambles (q,d,c) <- (32s+d, g, 256f+c).  DMA access
    patterns cannot gather strided partitions (rearrange-split APs
    read garbage), and per-group scatters waste ~0.7 us issue each.

The walrus build in this container accepts at most ONE sync wait per
instruction (setupSyncWait in CoreV3GenImpl), while Tile emits one wait
per dependent processor.  _split_multiwaits() post-processes the
finalized BIR: for every instruction with N>1 waits it inserts N-1
single-wait NOPs immediately before it on the same engine queue.
"""

import numpy as np
import ml_dtypes

import concourse.bass as bass
import concourse.mybir as mybir
import concourse.tile as tile
from concourse.bass import ds, ts
from concourse.bass_utils import run_bass_kernel_spmd

B, Q, K, D, C = 4, 128, 128, 3, 256
N_CORES = 8
QSH = Q // 2  # 64 q rows per core
QB = 8  # q values per ef tile (512 KiB DMA)
NG = QSH // QB
F32 = mybir.dt.float32
F16 = mybir.dt.bfloat16
BF16 = ml_dtypes.bfloat16

# mlp pack free-dim layout (fp32 cols per partition)
PK_W1 = 0          # [128, 2, 256]  W1[(o p), n] -> p, o, n
PK_W2 = 512        # [128, 2, 256]
PK_XT = 1024       # [128, 2, 128]  x^T[(o p), k] -> p, o, k
PK_B1 = 1280       # [1, 256] b1 on partition 0
PK_B2 = 1536       # [1, 256] b2 on partition 0
PK_F = 1792

# evmask pack free-dim layout
EM_EV = 0          # [128, 3, 64]   ev^T[k, d, q]
EM_MASK = 192      # [128, 64]      mask^T[k, q]
EM_F = 256

_NC_CACHE = {}


def _split_multiwaits(nc):
    """Legalize for the 1-sync-wait-per-instruction walrus: hoist all but
    the last wait of each instruction onto single-wait NOPs placed just
    before it on the same engine queue."""
    n = 0
    for f in nc.m.functions:
        for bb in f.blocks:
            out = []
            for inst in bb.instructions:
                si = inst.sync_info
                if si is not None and si.on_wait and len(si.on_wait) > 1:
                    waits = list(si.on_wait)
                    for w in waits[:-1]:
                        n += 1
                        nop = mybir.InstNoOp(
                            name=f"{inst.name}-wsplit{n}", ins=[], outs=[]
                        )
                        nop.engine = inst.engine
                        nop.sync_info = mybir.SyncInfo(on_wait=[w], on_update=[])
                        out.append(nop)
                    inst.sync_info = mybir.SyncInfo(
                        on_wait=[waits[-1]], on_update=list(si.on_update)
                    )
                out.append(inst)
            bb.instructions = out
    return nc


def _build_nc(split=True):
    nc = bass.Bass()

    ef_d = nc.declare_dram_parameter("efT", [K, QSH, C], F16, isOutput=False)
    mlp_d = nc.declare_dram_parameter("mlp", [128, PK_F], F32, isOutput=False)
    em_d = nc.declare_dram_parameter("evmask", [K, EM_F], F32, isOutput=False)
    out_d = nc.declare_dram_parameter("oraw", [4, D, NG, 2 * C], F32, isOutput=True)

    with tile.TileContext(nc) as tc:
        with (
            tc.tile_pool(name="const", bufs=1) as cpool,
            tc.tile_pool(name="efp", bufs=1) as efpool,
            tc.tile_pool(name="pprep", bufs=1, space="PSUM") as pprep,
            tc.tile_pool(name="pout", bufs=5, space="PSUM") as pout,
        ):
            # warm tile first (tiny gpsimd memset) so the shadow matmuls
            # can issue as soon as the PE preamble ends
            w_warm = cpool.tile([128, C], F16)
            nc.gpsimd.memset(w_warm[:], 0.0)
            ones_sb = cpool.tile([1, 128], F32)
            nc.gpsimd.memset(ones_sb[:], 1.0)

            # ---- input DMAs.  sync: weight pack, then ef groups in
            # consumption order.  scalar: ev/mask pack. ----
            mlp_sb = cpool.tile([128, PK_F], F32)
            nc.scalar.dma_start(mlp_sb[:], mlp_d[:, :])
            ef_slots = [
                efpool.tile([K, QB, C], F16, tag=f"ef{g}", name=f"ef{g}")
                for g in range(NG)
            ]
            for g in range(NG):
                nc.sync.dma_start(ef_slots[g][:], ef_d[:, ts(g, QB), :])
            em_sb = cpool.tile([K, EM_F], F32)
            nc.scalar.dma_start(em_sb[:], em_d[:, :])

            w1v = mlp_sb[:, PK_W1 : PK_W1 + 512].rearrange("p (o n) -> p o n", n=C)
            w2v = mlp_sb[:, PK_W2 : PK_W2 + 512].rearrange("p (o n) -> p o n", n=C)
            xTv = mlp_sb[:, PK_XT : PK_XT + 256].rearrange("p (o k) -> p o k", k=128)
            b1v = mlp_sb[0:1, PK_B1 : PK_B1 + C]
            b2v = mlp_sb[0:1, PK_B2 : PK_B2 + C]
            evTv = em_sb[:, EM_EV : EM_EV + 192].rearrange("p (d q) -> p d q", q=QSH)
            maskTv = em_sb[:, EM_MASK : EM_MASK + QSH]

            # ---- PE shadow warm-up: dep-free bf16 matmuls fill the PE
            # idle window while the weight pack lands; ~3 us of activity
            # flips the HAM throttle to 8/8 before the MLP needs it ----
            warm_ps = pout.tile([128, 2 * C], F32, tag="opsum", name="warm_ps")
            for _ in range(20):
                nc.tensor.matmul(
                    warm_ps[:, :C], w_warm[:, :128], w_warm[:, :C],
                    start=True, stop=True,
                )

            # ---- MLP (fp32), fully transpose-free ----
            h1T_ps = [
                pprep.tile([128, 128], F32, tag=f"prep{i}", name=f"h1T{i}")
                for i in range(2)
            ]
            for dh in range(2):
                nc.tensor.matmul(
                    h1T_ps[dh][:], w1v[:, 0, ts(dh, 128)], xTv[:, 0, :],
                    start=True, stop=False,
                )
                nc.tensor.matmul(
                    h1T_ps[dh][:], w1v[:, 1, ts(dh, 128)], xTv[:, 1, :],
                    start=False, stop=False,
                )
                nc.tensor.matmul(
                    h1T_ps[dh][:], b1v[:, ts(dh, 128)], ones_sb[:],
                    start=False, stop=True, tile_position=(0, 0),
                )
            sigT_sb = cpool.tile([128, 2, 128], F32)
            h1sT_sb = cpool.tile([128, 2, 128], F32)
            for dh in range(2):
                nc.scalar.activation(
                    sigT_sb[:, dh], h1T_ps[dh][:], mybir.ActivationFunctionType.Sigmoid
                )
                nc.vector.tensor_tensor(
                    h1sT_sb[:, dh], h1T_ps[dh][:], sigT_sb[:, dh], mybir.AluOpType.mult
                )
            h_ps = pprep.tile([128, C], F32, tag="hps", name="h_ps")
            nc.tensor.matmul(h_ps[:], h1sT_sb[:, 0], w2v[:, 0], start=True, stop=False)
            nc.tensor.matmul(h_ps[:], h1sT_sb[:, 1], w2v[:, 1], start=False, stop=False)
            # bias: h_ps[k, c] += ones[k] x b2[c]
            nc.tensor.matmul(
                h_ps[:], ones_sb[:], b2v[:], start=False, stop=True,
                tile_position=(0, 0),
            )

            # ---- w[k, q, 32] = (mask * ev)^T padded to 32 stationary
            # columns, built in fp32 then cast densely on the scalar
            # engine (strided bf16 DVE writes corrupt neighbor lanes,
            # and gpsimd casts are ~5x slower than ACT) ----
            w32_sb = cpool.tile([128, QSH, 32], F32)
            nc.vector.memset(w32_sb[:], 0.0)
            for d in range(D):
                nc.vector.tensor_copy(w32_sb[:, :, d], evTv[:, d, :])
            nc.vector.tensor_tensor(
                w32_sb[:, :, :D],
                w32_sb[:, :, :D],
                maskTv[:, :, None].to_broadcast([K, QSH, D]),
                mybir.AluOpType.mult,
            )
            w_sb = cpool.tile([128, QSH, 32], F16)
            nc.scalar.copy(out=w_sb[:], in_=w32_sb[:])

            # cast h -> bf16 and replicate x8 along q (doubling copies)
            h8_sb = cpool.tile([128, QB * C], F16)
            nc.scalar.copy(out=h8_sb[:, :C], in_=h_ps[:])
            nc.vector.tensor_copy(h8_sb[:, C : 2 * C], h8_sb[:, :C])
            nc.vector.tensor_copy(h8_sb[:, 2 * C : 4 * C], h8_sb[:, : 2 * C])
            nc.vector.tensor_copy(h8_sb[:, 4 * C : 8 * C], h8_sb[:, : 4 * C])

            # ---- main loop; all groups drain into one SBUF accumulator
            # and leave as a single DMA at the end ----
            o_all = cpool.tile([128, NG, 2 * C], F32)
            for g in range(NG):
                ef_t = ef_slots[g]
                halves = 2 if g == NG - 1 else 1
                ps = pout.tile([128, 2 * C], F32, tag="opsum", name="ps")
                for hv in range(halves):
                    js = range(hv * QB // halves, (hv + 1) * QB // halves)
                    nq = len(js)
                    nc.vector.tensor_tensor(
                        ef_t[:, js.start : js.stop, :],
                        ef_t[:, js.start : js.stop, :],
                        h8_sb[:, : nq * C].rearrange("p (j c) -> p j c", c=C),
                        mybir.AluOpType.mult,
                    )
                    for j in js:
                        f, s = j // 4, j % 4
                        q = g * QB + j
                        nc.tensor.matmul(
                            ps[ds(32 * s, 32), ds(C * f, C)],
                            w_sb[:, q, :],
                            ef_t[:, j, :],
                            start=True,
                            stop=True,
                            tile_position=(0, 32 * s),
                        )
                    nc.scalar.copy(
                        out=o_all[:, g, ds(hv * (2 * C) // halves, (2 * C) // halves)],
                        in_=ps[:, ds(hv * (2 * C) // halves, (2 * C) // halves)],
                    )
            # final drain to DRAM in d-major scratch layout: per s, one
            # DMA of 3 x 4 KiB dense descriptors.  The host unscrambles
            # out[8g+4f+s, d, c] = oraw[s, d, g, 256f+c] (free reshape).
            for s in range(4):
                eng = (nc.scalar, nc.scalar, nc.sync, nc.sync)[s]
                eng.dma_start(out_d[s], o_all[ds(32 * s, D), :, :])

    return _split_multiwaits(nc) if split else nc


def _get_nc():
    if "nc" not in _NC_CACHE:
        _NC_CACHE["nc"] = _build_nc()
    return _NC_CACHE["nc"]


def _in_maps(inputs):
    x = np.asarray(inputs["x"], dtype=np.float32)
    ev = np.asarray(inputs["ev"], dtype=np.float32)
    ef = np.asarray(inputs["ef"], dtype=np.float32)
    am = np.asarray(inputs["access_mask"], dtype=np.float32)
    W1 = np.asarray(inputs["W1"], dtype=np.float32)
    b1 = np.asarray(inputs["b1"], dtype=np.float32)
    W2 = np.asarray(inputs["W2"], dtype=np.float32)
    b2 = np.asarray(inputs["b2"], dtype=np.float32)

    # shared weight pack: [128, PK_F] fp32
    pack = np.zeros((128, PK_F), dtype=np.float32)
    pack[:, PK_W1 : PK_W1 + 512] = W1.reshape(2, 128, C).transpose(1, 0, 2).reshape(
        128, 512
    )
    pack[:, PK_W2 : PK_W2 + 512] = W2.reshape(2, 128, C).transpose(1, 0, 2).reshape(
        128, 512
    )
    pack[0, PK_B1 : PK_B1 + C] = b1
    pack[0, PK_B2 : PK_B2 + C] = b2

    maps = []
    for core in range(N_CORES):
        b, qh = core // 2, core % 2
        sl = slice(qh * QSH, (qh + 1) * QSH)
        pk = pack.copy()
        # xT: x[b]^T [C, K] -> [128, 2, 128] (p = c % 128, o = c // 128)
        pk[:, PK_XT : PK_XT + 256] = (
            x[b].T.reshape(2, 128, 128).transpose(1, 0, 2).reshape(128, 256)
        )
        em = np.empty((K, EM_F), dtype=np.float32)
        em[:, EM_EV : EM_EV + 192] = (
            ev[b, sl].transpose(1, 2, 0).reshape(K, 192)
        )
        em[:, EM_MASK : EM_MASK + QSH] = am[b, sl].T
        maps.append(
            {
                "efT": np.ascontiguousarray(
                    ef[b, sl].transpose(1, 0, 2).astype(BF16)
                ),
                "mlp": pk,
                "evmask": em,
            }
        )
    return maps


def _gather(results):
    out = np.empty((B, Q, D, C), dtype=np.float32)
    for core in range(N_CORES):
        b, qh = core // 2, core % 2
        oraw = results[core]["oraw"]  # [4, 3, NG, 512]
        arr = oraw.reshape(4, D, NG, 2, C)  # [s, d, g, f, c]
        out[b, qh * QSH : (qh + 1) * QSH] = (
            arr.transpose(2, 3, 0, 1, 4).reshape(QSH, D, C)
        )
    return out


def _run(inputs, trace=False, **kwargs):
    nc = _get_nc()
    res = run_bass_kernel_spmd(
        nc, _in_maps(inputs), list(range(N_CORES)), trace=trace, **kwargs
    )
    return _gather(res.results), res


def kernel(**inputs) -> np.ndarray:
    out, _ = _run(inputs, trace=False)
    return out


# revision 16
# speedup vs baseline: 1.1055x; 1.1055x over previous
"""Trainium2 Bass kernel for the fused GNN message-passing block.

Reference computation (per batch b):
    h = silu(x @ W1 + b1) @ W2 + b2                       # [K, C]
    out[q, d, c] = sum_k mask[q,k] * ev[q,k,d] * ef[q,k,c] * h[k,c]

Sharding: data-parallel over (b, q-half) -> 8 cores, each core handles
one b (of 4) and 64 of the 128 q values.  All large tensors carry the
leading b dim; the tiny MLP weights are replicated.

v3 layout/precision strategy:
  - ef is cast to bf16 on the host: halves the dominant HBM stream
    (8.4 -> 4.2 MB per core; one HWDGE queue sustains ~420 GB/s),
    doubles DVE throughput of the big ef*h multiply (2x 16-bit mode),
    and quadruples PE matmul column rate vs fp32.
  - The MLP runs in fp32; h is cast to bf16 and replicated x8 along q
    (log2 doubling copies) so the main multiply has unit-stride 16-bit
    operands.  w = (mask*ev)^T is built in fp32 (strided bf16 DVE
    writes corrupt neighboring 16-bit lanes) and cast densely on
    gpsimd.
  - HWDGE descriptor generation costs ~0.7 us of engine-queue time per
    dma_start, so the five MLP weight tensors ship as ONE packed param
    and ev/mask as another; ef streams as 8 x 512 KiB on the sync
    queue, in consumption order, behind the weight pack.
  - A burst of dep-free bf16 matmuls bridges the PE idle window while
    the weights land, so the HAM activity monitor reaches 8/8 before
    the MLP and main loop (cold PE runs at 1.2 GHz, warm at 2.4).
  - Outputs accumulate in SBUF and leave as ONE DMA at the end; the
    host unscrambles (q,d,c) <- (32s+d, g, 256f+c).  DMA access
    patterns cannot gather strided partitions (rearrange-split APs
    read garbage), and per-group scatters waste ~0.7 us issue each.

The walrus build in this container accepts at most ONE sync wait per
instruction (setupSyncWait in CoreV3GenImpl), while Tile emits one wait
per dependent processor.  _split_multiwaits() post-processes the
finalized BIR: for every instruction with N>1 waits it inserts N-1
single-wait NOPs immediately before it on the same engine queue.
"""

import numpy as np
import ml_dtypes

import concourse.bass as bass
import concourse.mybir as mybir
import concourse.tile as tile
from concourse.bass import ds, ts
from concourse.bass_utils import run_bass_kernel_spmd

B, Q, K, D, C = 4, 128, 128, 3, 256
N_CORES = 8
QSH = Q // 2  # 64 q rows per core
QB = 8  # q values per ef tile (512 KiB DMA)
NG = QSH // QB
F32 = mybir.dt.float32
F16 = mybir.dt.bfloat16
BF16 = ml_dtypes.bfloat16

# mlp pack free-dim layout (fp32 cols per partition)
PK_W1 = 0          # [128, 2, 256]  W1[(o p), n] -> p, o, n
PK_W2 = 512        # [128, 2, 256]
PK_XT = 1024       # [128, 2, 128]  x^T[(o p), k] -> p, o, k
PK_B1 = 1280       # [1, 256] b1 on partition 0
PK_B2 = 1536       # [1, 256] b2 on partition 0
PK_F = 1792

# evmask pack free-dim layout
EM_EV = 0          # [128, 3, 64]   ev^T[k, d, q]
EM_MASK = 192      # [128, 64]      mask^T[k, q]
EM_F = 256

_NC_CACHE = {}


def _split_multiwaits(nc):
    """Legalize for the 1-sync-wait-per-instruction walrus: hoist all but
    the last wait of each instruction onto single-wait NOPs placed just
    before it on the same engine queue."""
    n = 0
    for f in nc.m.functions:
        for bb in f.blocks:
            out = []
            for inst in bb.instructions:
                si = inst.sync_info
                if si is not None and si.on_wait and len(si.on_wait) > 1:
                    waits = list(si.on_wait)
                    for w in waits[:-1]:
                        n += 1
                        nop = mybir.InstNoOp(
                            name=f"{inst.name}-wsplit{n}", ins=[], outs=[]
                        )
                        nop.engine = inst.engine
                        nop.sync_info = mybir.SyncInfo(on_wait=[w], on_update=[])
                        out.append(nop)
                    inst.sync_info = mybir.SyncInfo(
                        on_wait=[waits[-1]], on_update=list(si.on_update)
                    )
                out.append(inst)
            bb.instructions = out
    return nc


def _build_nc(split=True):
    nc = bass.Bass()

    ef_d = nc.declare_dram_parameter("efT", [K, QSH, C], F16, isOutput=False)
    mlp_d = nc.declare_dram_parameter("mlp", [128, PK_F], F16, isOutput=False)
    em_d = nc.declare_dram_parameter("evmask", [K, EM_F], F32, isOutput=False)
    out_d = nc.declare_dram_parameter("oraw", [4, D, NG, 2 * C], F32, isOutput=True)

    with tile.TileContext(nc) as tc:
        with (
            tc.tile_pool(name="const", bufs=1) as cpool,
            tc.tile_pool(name="efp", bufs=1) as efpool,
            tc.tile_pool(name="pprep", bufs=1, space="PSUM") as pprep,
            tc.tile_pool(name="pout", bufs=5, space="PSUM") as pout,
        ):
            ones_sb = cpool.tile([1, 128], F16)
            nc.gpsimd.memset(ones_sb[:], 1.0)

            # ---- input DMAs.  sync: weight pack, then ef groups in
            # consumption order.  scalar: ev/mask pack. ----
            mlp_sb = cpool.tile([128, PK_F], F16)
            nc.scalar.dma_start(mlp_sb[:], mlp_d[:, :])
            ef_slots = [
                efpool.tile([K, QB, C], F16, tag=f"ef{g}", name=f"ef{g}")
                for g in range(NG)
            ]
            for g in range(NG):
                nc.sync.dma_start(ef_slots[g][:], ef_d[:, ts(g, QB), :])
            em_sb = cpool.tile([K, EM_F], F32)
            nc.scalar.dma_start(em_sb[:], em_d[:, :])

            w1v = mlp_sb[:, PK_W1 : PK_W1 + 512].rearrange("p (o n) -> p o n", n=C)
            w2v = mlp_sb[:, PK_W2 : PK_W2 + 512].rearrange("p (o n) -> p o n", n=C)
            xTv = mlp_sb[:, PK_XT : PK_XT + 256].rearrange("p (o k) -> p o k", k=128)
            b1v = mlp_sb[0:1, PK_B1 : PK_B1 + C]
            b2v = mlp_sb[0:1, PK_B2 : PK_B2 + C]
            evTv = em_sb[:, EM_EV : EM_EV + 192].rearrange("p (d q) -> p d q", q=QSH)
            maskTv = em_sb[:, EM_MASK : EM_MASK + QSH]

            # ---- MLP (fp32), fully transpose-free ----
            h1T_ps = [
                pprep.tile([128, 128], F32, tag=f"prep{i}", name=f"h1T{i}")
                for i in range(2)
            ]
            for dh in range(2):
                nc.tensor.matmul(
                    h1T_ps[dh][:], w1v[:, 0, ts(dh, 128)], xTv[:, 0, :],
                    start=True, stop=False,
                )
                nc.tensor.matmul(
                    h1T_ps[dh][:], w1v[:, 1, ts(dh, 128)], xTv[:, 1, :],
                    start=False, stop=False,
                )
                nc.tensor.matmul(
                    h1T_ps[dh][:], b1v[:, ts(dh, 128)], ones_sb[:],
                    start=False, stop=True, tile_position=(0, 0),
                )
            sigT_sb = cpool.tile([128, 2, 128], F32)
            h1sT_sb = cpool.tile([128, 2, 128], F16)
            for dh in range(2):
                nc.scalar.activation(
                    sigT_sb[:, dh], h1T_ps[dh][:], mybir.ActivationFunctionType.Sigmoid
                )
                nc.vector.tensor_tensor(
                    h1sT_sb[:, dh], h1T_ps[dh][:], sigT_sb[:, dh], mybir.AluOpType.mult
                )
            h_ps = pprep.tile([128, C], F32, tag="hps", name="h_ps")
            nc.tensor.matmul(h_ps[:], h1sT_sb[:, 0], w2v[:, 0], start=True, stop=False)
            nc.tensor.matmul(h_ps[:], h1sT_sb[:, 1], w2v[:, 1], start=False, stop=False)
            # bias: h_ps[k, c] += ones[k] x b2[c]
            nc.tensor.matmul(
                h_ps[:], ones_sb[:], b2v[:], start=False, stop=True,
                tile_position=(0, 0),
            )

            # ---- w[k, q, 32] = (mask * ev)^T padded to 32 stationary
            # columns, built in fp32 then cast densely on the scalar
            # engine (strided bf16 DVE writes corrupt neighbor lanes,
            # and gpsimd casts are ~5x slower than ACT) ----
            w32_sb = cpool.tile([128, QSH, 32], F32)
            nc.vector.memset(w32_sb[:], 0.0)
            for d in range(D):
                nc.vector.tensor_copy(w32_sb[:, :, d], evTv[:, d, :])
            nc.vector.tensor_tensor(
                w32_sb[:, :, :D],
                w32_sb[:, :, :D],
                maskTv[:, :, None].to_broadcast([K, QSH, D]),
                mybir.AluOpType.mult,
            )
            w_sb = cpool.tile([128, QSH, 32], F16)
            nc.scalar.copy(out=w_sb[:], in_=w32_sb[:])

            # cast h -> bf16 and replicate x8 along q (doubling copies)
            h8_sb = cpool.tile([128, QB * C], F16)
            nc.scalar.copy(out=h8_sb[:, :C], in_=h_ps[:])
            nc.vector.tensor_copy(h8_sb[:, C : 2 * C], h8_sb[:, :C])
            nc.vector.tensor_copy(h8_sb[:, 2 * C : 4 * C], h8_sb[:, : 2 * C])
            nc.vector.tensor_copy(h8_sb[:, 4 * C : 8 * C], h8_sb[:, : 4 * C])

            # ---- main loop; all groups drain into one SBUF accumulator
            # and leave as a single DMA at the end ----
            o_all = cpool.tile([128, NG, 2 * C], F32)
            for g in range(NG):
                ef_t = ef_slots[g]
                halves = 2 if g == NG - 1 else 1
                ps = pout.tile([128, 2 * C], F32, tag="opsum", name="ps")
                for hv in range(halves):
                    js = range(hv * QB // halves, (hv + 1) * QB // halves)
                    nq = len(js)
                    nc.vector.tensor_tensor(
                        ef_t[:, js.start : js.stop, :],
                        ef_t[:, js.start : js.stop, :],
                        h8_sb[:, : nq * C].rearrange("p (j c) -> p j c", c=C),
                        mybir.AluOpType.mult,
                    )
                    for j in js:
                        f, s = j // 4, j % 4
                        q = g * QB + j
                        nc.tensor.matmul(
                            ps[ds(32 * s, 32), ds(C * f, C)],
                            w_sb[:, q, :],
                            ef_t[:, j, :],
                            start=True,
                            stop=True,
                            tile_position=(0, 32 * s),
                        )
                    nc.scalar.copy(
                        out=o_all[:, g, ds(hv * (2 * C) // halves, (2 * C) // halves)],
                        in_=ps[:, ds(hv * (2 * C) // halves, (2 * C) // halves)],
                    )
            # final drain to DRAM in d-major scratch layout: per s, one
            # DMA of 3 x 4 KiB dense descriptors.  The host unscrambles
            # out[8g+4f+s, d, c] = oraw[s, d, g, 256f+c] (free reshape).
            for s in range(4):
                eng = (nc.scalar, nc.scalar, nc.sync, nc.sync)[s]
                eng.dma_start(out_d[s], o_all[ds(32 * s, D), :, :])

    return _split_multiwaits(nc) if split else nc


def _get_nc():
    if "nc" not in _NC_CACHE:
        _NC_CACHE["nc"] = _build_nc()
    return _NC_CACHE["nc"]


def _in_maps(inputs):
    x = np.asarray(inputs["x"], dtype=np.float32)
    ev = np.asarray(inputs["ev"], dtype=np.float32)
    ef = np.asarray(inputs["ef"], dtype=np.float32)
    am = np.asarray(inputs["access_mask"], dtype=np.float32)
    W1 = np.asarray(inputs["W1"], dtype=np.float32)
    b1 = np.asarray(inputs["b1"], dtype=np.float32)
    W2 = np.asarray(inputs["W2"], dtype=np.float32)
    b2 = np.asarray(inputs["b2"], dtype=np.float32)

    # shared weight pack: [128, PK_F] fp32
    pack = np.zeros((128, PK_F), dtype=np.float32)  # cast to bf16 at the end
    pack[:, PK_W1 : PK_W1 + 512] = W1.reshape(2, 128, C).transpose(1, 0, 2).reshape(
        128, 512
    )
    pack[:, PK_W2 : PK_W2 + 512] = W2.reshape(2, 128, C).transpose(1, 0, 2).reshape(
        128, 512
    )
    pack[0, PK_B1 : PK_B1 + C] = b1
    pack[0, PK_B2 : PK_B2 + C] = b2

    maps = []
    for core in range(N_CORES):
        b, qh = core // 2, core % 2
        sl = slice(qh * QSH, (qh + 1) * QSH)
        pk = pack.copy()
        # xT: x[b]^T [C, K] -> [128, 2, 128] (p = c % 128, o = c // 128)
        pk[:, PK_XT : PK_XT + 256] = (
            x[b].T.reshape(2, 128, 128).transpose(1, 0, 2).reshape(128, 256)
        )
        em = np.empty((K, EM_F), dtype=np.float32)
        em[:, EM_EV : EM_EV + 192] = (
            ev[b, sl].transpose(1, 2, 0).reshape(K, 192)
        )
        em[:, EM_MASK : EM_MASK + QSH] = am[b, sl].T
        maps.append(
            {
                "efT": np.ascontiguousarray(
                    ef[b, sl].transpose(1, 0, 2).astype(BF16)
                ),
                "mlp": pk.astype(BF16),
                "evmask": em,
            }
        )
    return maps


def _gather(results):
    out = np.empty((B, Q, D, C), dtype=np.float32)
    for core in range(N_CORES):
        b, qh = core // 2, core % 2
        oraw = results[core]["oraw"]  # [4, 3, NG, 512]
        arr = oraw.reshape(4, D, NG, 2, C)  # [s, d, g, f, c]
        out[b, qh * QSH : (qh + 1) * QSH] = (
            arr.transpose(2, 3, 0, 1, 4).reshape(QSH, D, C)
        )
    return out


def _run(inputs, trace=False, **kwargs):
    nc = _get_nc()
    res = run_bass_kernel_spmd(
        nc, _in_maps(inputs), list(range(N_CORES)), trace=trace, **kwargs
    )
    return _gather(res.results), res


def kernel(**inputs) -> np.ndarray:
    out, _ = _run(inputs, trace=False)
    return out


# revision 17
# speedup vs baseline: 1.1626x; 1.0516x over previous
"""Trainium2 Bass kernel for the fused GNN message-passing block.

Reference computation (per batch b):
    h = silu(x @ W1 + b1) @ W2 + b2                       # [K, C]
    out[q, d, c] = sum_k mask[q,k] * ev[q,k,d] * ef[q,k,c] * h[k,c]

Sharding: data-parallel over (b, q-half) -> 8 cores, each core handles
one b (of 4) and 64 of the 128 q values.  All large tensors carry the
leading b dim; the tiny MLP weights are replicated.

v3 layout/precision strategy:
  - ef is cast to bf16 on the host: halves the dominant HBM stream
    (8.4 -> 4.2 MB per core; one HWDGE queue sustains ~420 GB/s),
    doubles DVE throughput of the big ef*h multiply (2x 16-bit mode),
    and quadruples PE matmul column rate vs fp32.
  - The MLP runs in fp32; h is cast to bf16 and replicated x8 along q
    (log2 doubling copies) so the main multiply has unit-stride 16-bit
    operands.  w = (mask*ev)^T is built in fp32 (strided bf16 DVE
    writes corrupt neighboring 16-bit lanes) and cast densely on
    gpsimd.
  - HWDGE descriptor generation costs ~0.7 us of engine-queue time per
    dma_start, so the five MLP weight tensors ship as ONE packed param
    and ev/mask as another; ef streams as 8 x 512 KiB on the sync
    queue, in consumption order, behind the weight pack.
  - A burst of dep-free bf16 matmuls bridges the PE idle window while
    the weights land, so the HAM activity monitor reaches 8/8 before
    the MLP and main loop (cold PE runs at 1.2 GHz, warm at 2.4).
  - Outputs accumulate in SBUF and leave as ONE DMA at the end; the
    host unscrambles (q,d,c) <- (32s+d, g, 256f+c).  DMA access
    patterns cannot gather strided partitions (rearrange-split APs
    read garbage), and per-group scatters waste ~0.7 us issue each.

The walrus build in this container accepts at most ONE sync wait per
instruction (setupSyncWait in CoreV3GenImpl), while Tile emits one wait
per dependent processor.  _split_multiwaits() post-processes the
finalized BIR: for every instruction with N>1 waits it inserts N-1
single-wait NOPs immediately before it on the same engine queue.
"""

import numpy as np
import ml_dtypes

import concourse.bass as bass
import concourse.mybir as mybir
import concourse.tile as tile
from concourse.bass import ds, ts
from concourse.bass_utils import run_bass_kernel_spmd

B, Q, K, D, C = 4, 128, 128, 3, 256
N_CORES = 8
QSH = Q // 2  # 64 q rows per core
QB = 8  # q values per ef tile (512 KiB DMA)
NG = QSH // QB
F32 = mybir.dt.float32
F16 = mybir.dt.bfloat16
BF16 = ml_dtypes.bfloat16

# mlp pack free-dim layout (fp32 cols per partition)
PK_W1 = 0          # [128, 2, 256]  W1[(o p), n] -> p, o, n
PK_W2 = 512        # [128, 2, 256]
PK_XT = 1024       # [128, 2, 128]  x^T[(o p), k] -> p, o, k
PK_B1 = 1280       # [1, 256] b1 on partition 0
PK_B2 = 1536       # [1, 256] b2 on partition 0
PK_F = 1792

# evmask pack free-dim layout
EM_EV = 0          # [128, 3, 64]   ev^T[k, d, q]
EM_MASK = 192      # [128, 64]      mask^T[k, q]
EM_F = 256

_NC_CACHE = {}


def _split_multiwaits(nc):
    """Legalize for the 1-sync-wait-per-instruction walrus: hoist all but
    the last wait of each instruction onto single-wait NOPs placed just
    before it on the same engine queue."""
    n = 0
    for f in nc.m.functions:
        for bb in f.blocks:
            out = []
            for inst in bb.instructions:
                si = inst.sync_info
                if si is not None and si.on_wait and len(si.on_wait) > 1:
                    waits = list(si.on_wait)
                    for w in waits[:-1]:
                        n += 1
                        nop = mybir.InstNoOp(
                            name=f"{inst.name}-wsplit{n}", ins=[], outs=[]
                        )
                        nop.engine = inst.engine
                        nop.sync_info = mybir.SyncInfo(on_wait=[w], on_update=[])
                        out.append(nop)
                    inst.sync_info = mybir.SyncInfo(
                        on_wait=[waits[-1]], on_update=list(si.on_update)
                    )
                out.append(inst)
            bb.instructions = out
    return nc


def _build_nc(split=True):
    nc = bass.Bass()

    ef_d = nc.declare_dram_parameter("efT", [K, QSH, C], F16, isOutput=False)
    mlp_d = nc.declare_dram_parameter("mlp", [128, PK_F], F16, isOutput=False)
    em_d = nc.declare_dram_parameter("evmask", [K, EM_F], F32, isOutput=False)
    out_d = nc.declare_dram_parameter("oraw", [4, D, NG, 2 * C], F16, isOutput=True)

    with tile.TileContext(nc) as tc:
        with (
            tc.tile_pool(name="const", bufs=1) as cpool,
            tc.tile_pool(name="efp", bufs=1) as efpool,
            tc.tile_pool(name="pprep", bufs=1, space="PSUM") as pprep,
            tc.tile_pool(name="pout", bufs=5, space="PSUM") as pout,
        ):
            ones_sb = cpool.tile([1, 128], F16)
            nc.gpsimd.memset(ones_sb[:], 1.0)

            # ---- input DMAs.  sync: weight pack, then ef groups in
            # consumption order.  scalar: ev/mask pack. ----
            em_sb = cpool.tile([K, EM_F], F32)
            nc.scalar.dma_start(em_sb[:], em_d[:, :])
            mlp_sb = cpool.tile([128, PK_F], F16)
            nc.scalar.dma_start(mlp_sb[:], mlp_d[:, :])
            ef_slots = [
                efpool.tile([K, QB, C], F16, tag=f"ef{g}", name=f"ef{g}")
                for g in range(NG)
            ]
            for g in range(NG):
                nc.sync.dma_start(ef_slots[g][:], ef_d[:, ts(g, QB), :])

            w1v = mlp_sb[:, PK_W1 : PK_W1 + 512].rearrange("p (o n) -> p o n", n=C)
            w2v = mlp_sb[:, PK_W2 : PK_W2 + 512].rearrange("p (o n) -> p o n", n=C)
            xTv = mlp_sb[:, PK_XT : PK_XT + 256].rearrange("p (o k) -> p o k", k=128)
            b1v = mlp_sb[0:1, PK_B1 : PK_B1 + C]
            b2v = mlp_sb[0:1, PK_B2 : PK_B2 + C]
            evTv = em_sb[:, EM_EV : EM_EV + 192].rearrange("p (d q) -> p d q", q=QSH)
            maskTv = em_sb[:, EM_MASK : EM_MASK + QSH]

            # ---- MLP (fp32), fully transpose-free ----
            h1T_ps = [
                pprep.tile([128, 128], F32, tag=f"prep{i}", name=f"h1T{i}")
                for i in range(2)
            ]
            for dh in range(2):
                nc.tensor.matmul(
                    h1T_ps[dh][:], w1v[:, 0, ts(dh, 128)], xTv[:, 0, :],
                    start=True, stop=False,
                )
                nc.tensor.matmul(
                    h1T_ps[dh][:], w1v[:, 1, ts(dh, 128)], xTv[:, 1, :],
                    start=False, stop=False,
                )
                nc.tensor.matmul(
                    h1T_ps[dh][:], b1v[:, ts(dh, 128)], ones_sb[:],
                    start=False, stop=True, tile_position=(0, 0),
                )
            sigT_sb = cpool.tile([128, 2, 128], F32)
            h1sT_sb = cpool.tile([128, 2, 128], F16)
            for dh in range(2):
                nc.scalar.activation(
                    sigT_sb[:, dh], h1T_ps[dh][:], mybir.ActivationFunctionType.Sigmoid
                )
                nc.vector.tensor_tensor(
                    h1sT_sb[:, dh], h1T_ps[dh][:], sigT_sb[:, dh], mybir.AluOpType.mult
                )
            h_ps = pprep.tile([128, C], F32, tag="hps", name="h_ps")
            nc.tensor.matmul(h_ps[:], h1sT_sb[:, 0], w2v[:, 0], start=True, stop=False)
            nc.tensor.matmul(h_ps[:], h1sT_sb[:, 1], w2v[:, 1], start=False, stop=False)
            # bias: h_ps[k, c] += ones[k] x b2[c]
            nc.tensor.matmul(
                h_ps[:], ones_sb[:], b2v[:], start=False, stop=True,
                tile_position=(0, 0),
            )

            # ---- w[k, q, 32] = (mask * ev)^T padded to 32 stationary
            # columns, built in fp32 then cast densely on the scalar
            # engine (strided bf16 DVE writes corrupt neighbor lanes,
            # and gpsimd casts are ~5x slower than ACT) ----
            w32_sb = cpool.tile([128, QSH, 32], F32)
            nc.vector.memset(w32_sb[:], 0.0)
            for d in range(D):
                nc.vector.tensor_copy(w32_sb[:, :, d], evTv[:, d, :])
            nc.vector.tensor_tensor(
                w32_sb[:, :, :D],
                w32_sb[:, :, :D],
                maskTv[:, :, None].to_broadcast([K, QSH, D]),
                mybir.AluOpType.mult,
            )
            w_sb = cpool.tile([128, QSH, 32], F16)
            nc.scalar.copy(out=w_sb[:], in_=w32_sb[:])

            # cast h -> bf16 and replicate x8 along q (doubling copies)
            h8_sb = cpool.tile([128, QB * C], F16)
            nc.scalar.copy(out=h8_sb[:, :C], in_=h_ps[:])
            nc.vector.tensor_copy(h8_sb[:, C : 2 * C], h8_sb[:, :C])
            nc.vector.tensor_copy(h8_sb[:, 2 * C : 4 * C], h8_sb[:, : 2 * C])
            nc.vector.tensor_copy(h8_sb[:, 4 * C : 8 * C], h8_sb[:, : 4 * C])

            # ---- main loop; all groups drain into one SBUF accumulator
            # and leave as a single DMA at the end ----
            o_all = cpool.tile([128, NG, 2 * C], F16)
            for g in range(NG):
                ef_t = ef_slots[g]
                halves = 2 if g == NG - 1 else 1
                ps = pout.tile([128, 2 * C], F32, tag="opsum", name="ps")
                for hv in range(halves):
                    js = range(hv * QB // halves, (hv + 1) * QB // halves)
                    nq = len(js)
                    nc.vector.tensor_tensor(
                        ef_t[:, js.start : js.stop, :],
                        ef_t[:, js.start : js.stop, :],
                        h8_sb[:, : nq * C].rearrange("p (j c) -> p j c", c=C),
                        mybir.AluOpType.mult,
                    )
                    for j in js:
                        f, s = j // 4, j % 4
                        q = g * QB + j
                        nc.tensor.matmul(
                            ps[ds(32 * s, 32), ds(C * f, C)],
                            w_sb[:, q, :],
                            ef_t[:, j, :],
                            start=True,
                            stop=True,
                            tile_position=(0, 32 * s),
                        )
                    nc.scalar.copy(
                        out=o_all[:, g, ds(hv * (2 * C) // halves, (2 * C) // halves)],
                        in_=ps[:, ds(hv * (2 * C) // halves, (2 * C) // halves)],
                    )
                # stream finished output phases to DRAM while the loop
                # runs; HBM writes crawl (~30-50 GB/s effective), so only
                # the last group's phase sits on the tail.
                if g in (3, 6, 7):
                    glo = {3: 0, 6: 4, 7: 7}[g]
                    for s in range(4):
                        eng = (nc.sync, nc.sync, nc.scalar, nc.scalar)[s]
                        eng.dma_start(
                            out_d[s, :, glo : g + 1, :],
                            o_all[ds(32 * s, D), glo : g + 1, :],
                        )


    return _split_multiwaits(nc) if split else nc


def _get_nc():
    if "nc" not in _NC_CACHE:
        _NC_CACHE["nc"] = _build_nc()
    return _NC_CACHE["nc"]


def _in_maps(inputs):
    x = np.asarray(inputs["x"], dtype=np.float32)
    ev = np.asarray(inputs["ev"], dtype=np.float32)
    ef = np.asarray(inputs["ef"], dtype=np.float32)
    am = np.asarray(inputs["access_mask"], dtype=np.float32)
    W1 = np.asarray(inputs["W1"], dtype=np.float32)
    b1 = np.asarray(inputs["b1"], dtype=np.float32)
    W2 = np.asarray(inputs["W2"], dtype=np.float32)
    b2 = np.asarray(inputs["b2"], dtype=np.float32)

    # shared weight pack: [128, PK_F] fp32
    pack = np.zeros((128, PK_F), dtype=np.float32)  # cast to bf16 at the end
    pack[:, PK_W1 : PK_W1 + 512] = W1.reshape(2, 128, C).transpose(1, 0, 2).reshape(
        128, 512
    )
    pack[:, PK_W2 : PK_W2 + 512] = W2.reshape(2, 128, C).transpose(1, 0, 2).reshape(
        128, 512
    )
    pack[0, PK_B1 : PK_B1 + C] = b1
    pack[0, PK_B2 : PK_B2 + C] = b2

    maps = []
    for core in range(N_CORES):
        b, qh = core // 2, core % 2
        sl = slice(qh * QSH, (qh + 1) * QSH)
        pk = pack.copy()
        # xT: x[b]^T [C, K] -> [128, 2, 128] (p = c % 128, o = c // 128)
        pk[:, PK_XT : PK_XT + 256] = (
            x[b].T.reshape(2, 128, 128).transpose(1, 0, 2).reshape(128, 256)
        )
        em = np.empty((K, EM_F), dtype=np.float32)
        em[:, EM_EV : EM_EV + 192] = (
            ev[b, sl].transpose(1, 2, 0).reshape(K, 192)
        )
        em[:, EM_MASK : EM_MASK + QSH] = am[b, sl].T
        maps.append(
            {
                "efT": np.ascontiguousarray(
                    ef[b, sl].transpose(1, 0, 2).astype(BF16)
                ),
                "mlp": pk.astype(BF16),
                "evmask": em,
            }
        )
    return maps


def _gather(results):
    out = np.empty((B, Q, D, C), dtype=np.float32)
    for core in range(N_CORES):
        b, qh = core // 2, core % 2
        oraw = results[core]["oraw"].astype(np.float32)  # [4, 3, NG, 512]
        arr = oraw.reshape(4, D, NG, 2, C)  # [s, d, g, f, c]
        out[b, qh * QSH : (qh + 1) * QSH] = (
            arr.transpose(2, 3, 0, 1, 4).reshape(QSH, D, C)
        )
    return out


def _run(inputs, trace=False, **kwargs):
    nc = _get_nc()
    res = run_bass_kernel_spmd(
        nc, _in_maps(inputs), list(range(N_CORES)), trace=trace, **kwargs
    )
    return _gather(res.results), res


def kernel(**inputs) -> np.ndarray:
    out, _ = _run(inputs, trace=False)
    return out


# revision 18
# speedup vs baseline: 1.1657x; 1.0027x over previous
"""Trainium2 Bass kernel for the fused GNN message-passing block.

Reference computation (per batch b):
    h = silu(x @ W1 + b1) @ W2 + b2                       # [K, C]
    out[q, d, c] = sum_k mask[q,k] * ev[q,k,d] * ef[q,k,c] * h[k,c]

Sharding: data-parallel over (b, q-half) -> 8 cores, each core handles
one b (of 4) and 64 of the 128 q values.  All large tensors carry the
leading b dim; the tiny MLP weights are replicated.

v3 layout/precision strategy:
  - ef is cast to bf16 on the host: halves the dominant HBM stream
    (8.4 -> 4.2 MB per core; one HWDGE queue sustains ~420 GB/s),
    doubles DVE throughput of the big ef*h multiply (2x 16-bit mode),
    and quadruples PE matmul column rate vs fp32.
  - The MLP runs in fp32; h is cast to bf16 and replicated x8 along q
    (log2 doubling copies) so the main multiply has unit-stride 16-bit
    operands.  w = (mask*ev)^T is built in fp32 (strided bf16 DVE
    writes corrupt neighboring 16-bit lanes) and cast densely on
    gpsimd.
  - HWDGE descriptor generation costs ~0.7 us of engine-queue time per
    dma_start, so the five MLP weight tensors ship as ONE packed param
    and ev/mask as another; ef streams as 8 x 512 KiB on the sync
    queue, in consumption order, behind the weight pack.
  - A burst of dep-free bf16 matmuls bridges the PE idle window while
    the weights land, so the HAM activity monitor reaches 8/8 before
    the MLP and main loop (cold PE runs at 1.2 GHz, warm at 2.4).
  - Outputs accumulate in SBUF and leave as ONE DMA at the end; the
    host unscrambles (q,d,c) <- (32s+d, g, 256f+c).  DMA access
    patterns cannot gather strided partitions (rearrange-split APs
    read garbage), and per-group scatters waste ~0.7 us issue each.

The walrus build in this container accepts at most ONE sync wait per
instruction (setupSyncWait in CoreV3GenImpl), while Tile emits one wait
per dependent processor.  _split_multiwaits() post-processes the
finalized BIR: for every instruction with N>1 waits it inserts N-1
single-wait NOPs immediately before it on the same engine queue.
"""

import numpy as np
import ml_dtypes

import concourse.bass as bass
import concourse.mybir as mybir
import concourse.tile as tile
from concourse.bass import ds, ts
from concourse.bass_utils import run_bass_kernel_spmd

B, Q, K, D, C = 4, 128, 128, 3, 256
N_CORES = 8
QSH = Q // 2  # 64 q rows per core
QB = 8  # q values per ef tile (512 KiB DMA)
NG = QSH // QB
F32 = mybir.dt.float32
F16 = mybir.dt.bfloat16
BF16 = ml_dtypes.bfloat16

# mlp pack free-dim layout (fp32 cols per partition)
PK_W1 = 0          # [128, 2, 256]  W1[(o p), n] -> p, o, n
PK_W2 = 512        # [128, 2, 256]
PK_XT = 1024       # [128, 2, 128]  x^T[(o p), k] -> p, o, k
PK_B1 = 1280       # [1, 256] b1 on partition 0
PK_B2 = 1536       # [1, 256] b2 on partition 0
PK_F = 1792

# evmask pack free-dim layout
EM_EV = 0          # [128, 3, 64]   ev^T[k, d, q]
EM_MASK = 192      # [128, 64]      mask^T[k, q]
EM_F = 256

_NC_CACHE = {}


def _split_multiwaits(nc):
    """Legalize for the 1-sync-wait-per-instruction walrus: hoist all but
    the last wait of each instruction onto single-wait NOPs placed just
    before it on the same engine queue."""
    n = 0
    for f in nc.m.functions:
        for bb in f.blocks:
            out = []
            for inst in bb.instructions:
                si = inst.sync_info
                if si is not None and si.on_wait and len(si.on_wait) > 1:
                    waits = list(si.on_wait)
                    for w in waits[:-1]:
                        n += 1
                        nop = mybir.InstNoOp(
                            name=f"{inst.name}-wsplit{n}", ins=[], outs=[]
                        )
                        nop.engine = inst.engine
                        nop.sync_info = mybir.SyncInfo(on_wait=[w], on_update=[])
                        out.append(nop)
                    inst.sync_info = mybir.SyncInfo(
                        on_wait=[waits[-1]], on_update=list(si.on_update)
                    )
                out.append(inst)
            bb.instructions = out
    return nc


def _build_nc(split=True):
    nc = bass.Bass()

    ef_d = nc.declare_dram_parameter("efT", [K, QSH, C], F16, isOutput=False)
    mlp_d = nc.declare_dram_parameter("mlp", [128, PK_F], F16, isOutput=False)
    em_d = nc.declare_dram_parameter("evmask", [K, EM_F], F32, isOutput=False)
    out_d = nc.declare_dram_parameter("oraw", [4, D, NG, 2 * C], F16, isOutput=True)

    with tile.TileContext(nc) as tc:
        with (
            tc.tile_pool(name="const", bufs=1) as cpool,
            tc.tile_pool(name="efp", bufs=1) as efpool,
            tc.tile_pool(name="pprep", bufs=1, space="PSUM") as pprep,
            tc.tile_pool(name="pout", bufs=5, space="PSUM") as pout,
        ):
            ones_sb = cpool.tile([1, 128], F16)
            nc.gpsimd.memset(ones_sb[:], 1.0)

            # ---- input DMAs.  sync: weight pack, then ef groups in
            # consumption order.  scalar: ev/mask pack. ----
            em_sb = cpool.tile([K, EM_F], F32)
            nc.scalar.dma_start(em_sb[:], em_d[:, :])
            mlp_sb = cpool.tile([128, PK_F], F16)
            nc.scalar.dma_start(mlp_sb[:], mlp_d[:, :])

            w1v = mlp_sb[:, PK_W1 : PK_W1 + 512].rearrange("p (o n) -> p o n", n=C)
            w2v = mlp_sb[:, PK_W2 : PK_W2 + 512].rearrange("p (o n) -> p o n", n=C)
            xTv = mlp_sb[:, PK_XT : PK_XT + 256].rearrange("p (o k) -> p o k", k=128)
            b1v = mlp_sb[0:1, PK_B1 : PK_B1 + C]
            b2v = mlp_sb[0:1, PK_B2 : PK_B2 + C]
            evTv = em_sb[:, EM_EV : EM_EV + 192].rearrange("p (d q) -> p d q", q=QSH)
            maskTv = em_sb[:, EM_MASK : EM_MASK + QSH]

            # ---- w32 = (mask * ev)^T in fp32 (DVE, needs only evmask;
            # strided bf16 DVE writes corrupt neighbor lanes) ----
            w32_sb = cpool.tile([128, QSH, 32], F32)
            nc.vector.memset(w32_sb[:], 0.0)
            for d in range(D):
                nc.vector.tensor_copy(w32_sb[:, :, d], evTv[:, d, :])
            nc.vector.tensor_tensor(
                w32_sb[:, :, :D],
                w32_sb[:, :, :D],
                maskTv[:, :, None].to_broadcast([K, QSH, D]),
                mybir.AluOpType.mult,
            )

            # ---- MLP (fp32), fully transpose-free ----
            h1T_ps = [
                pprep.tile([128, 128], F32, tag=f"prep{i}", name=f"h1T{i}")
                for i in range(2)
            ]
            for dh in range(2):
                nc.tensor.matmul(
                    h1T_ps[dh][:], w1v[:, 0, ts(dh, 128)], xTv[:, 0, :],
                    start=True, stop=False,
                )
                nc.tensor.matmul(
                    h1T_ps[dh][:], w1v[:, 1, ts(dh, 128)], xTv[:, 1, :],
                    start=False, stop=False,
                )
                nc.tensor.matmul(
                    h1T_ps[dh][:], b1v[:, ts(dh, 128)], ones_sb[:],
                    start=False, stop=True, tile_position=(0, 0),
                )
            sigT_sb = cpool.tile([128, 2, 128], F32)
            h1sT_sb = cpool.tile([128, 2, 128], F16)
            for dh in range(2):
                nc.scalar.activation(
                    sigT_sb[:, dh], h1T_ps[dh][:], mybir.ActivationFunctionType.Sigmoid
                )
                nc.vector.tensor_tensor(
                    h1sT_sb[:, dh], h1T_ps[dh][:], sigT_sb[:, dh], mybir.AluOpType.mult
                )
            h_ps = pprep.tile([128, C], F32, tag="hps", name="h_ps")
            nc.tensor.matmul(h_ps[:], h1sT_sb[:, 0], w2v[:, 0], start=True, stop=False)
            nc.tensor.matmul(h_ps[:], h1sT_sb[:, 1], w2v[:, 1], start=False, stop=False)
            # bias: h_ps[k, c] += ones[k] x b2[c]
            nc.tensor.matmul(
                h_ps[:], ones_sb[:], b2v[:], start=False, stop=True,
                tile_position=(0, 0),
            )

            # cast h -> bf16 and replicate x8 along q (doubling copies)
            h8_sb = cpool.tile([128, QB * C], F16)
            nc.scalar.copy(out=h8_sb[:, :C], in_=h_ps[:])
            nc.vector.tensor_copy(h8_sb[:, C : 2 * C], h8_sb[:, :C])
            nc.vector.tensor_copy(h8_sb[:, 2 * C : 4 * C], h8_sb[:, : 2 * C])
            nc.vector.tensor_copy(h8_sb[:, 4 * C : 8 * C], h8_sb[:, : 4 * C])

            # dense fp32->bf16 cast of the stationary weights on ACT,
            # after the h8 seed so it never delays the first TT; the
            # first matmuls briefly trail the TTs instead.
            w_sb = cpool.tile([128, QSH, 32], F16)
            nc.scalar.copy(out=w_sb[:], in_=w32_sb[:])

            # ef streams on the otherwise-idle sync queue.  Emitted after
            # the MLP so DMA-completion semaphore lane thresholds for the
            # weight pack never include ef completions (the sync queue
            # itself starts these issues right after its preamble).
            ef_slots = [
                efpool.tile([K, QB, C], F16, tag=f"ef{g}", name=f"ef{g}")
                for g in range(NG)
            ]
            for g in range(NG):
                nc.sync.dma_start(ef_slots[g][:], ef_d[:, ts(g, QB), :])

            # ---- main loop; all groups drain into one SBUF accumulator
            # and leave as a single DMA at the end ----
            o_all = cpool.tile([128, NG, 2 * C], F16)
            for g in range(NG):
                ef_t = ef_slots[g]
                halves = 2 if g == NG - 1 else 1
                ps = pout.tile([128, 2 * C], F32, tag="opsum", name="ps")
                for hv in range(halves):
                    js = range(hv * QB // halves, (hv + 1) * QB // halves)
                    nq = len(js)
                    nc.vector.tensor_tensor(
                        ef_t[:, js.start : js.stop, :],
                        ef_t[:, js.start : js.stop, :],
                        h8_sb[:, : nq * C].rearrange("p (j c) -> p j c", c=C),
                        mybir.AluOpType.mult,
                    )
                    for j in js:
                        f, s = j // 4, j % 4
                        q = g * QB + j
                        nc.tensor.matmul(
                            ps[ds(32 * s, 32), ds(C * f, C)],
                            w_sb[:, q, :],
                            ef_t[:, j, :],
                            start=True,
                            stop=True,
                            tile_position=(0, 32 * s),
                        )
                    nc.scalar.copy(
                        out=o_all[:, g, ds(hv * (2 * C) // halves, (2 * C) // halves)],
                        in_=ps[:, ds(hv * (2 * C) // halves, (2 * C) // halves)],
                    )
                # stream finished output phases to DRAM while the loop
                # runs; HBM writes crawl (~30-50 GB/s effective), so only
                # the last group's phase sits on the tail.
                if g in (3, 6, 7):
                    glo = {3: 0, 6: 4, 7: 7}[g]
                    for s in range(4):
                        eng = (nc.sync, nc.sync, nc.scalar, nc.scalar)[s]
                        eng.dma_start(
                            out_d[s, :, glo : g + 1, :],
                            o_all[ds(32 * s, D), glo : g + 1, :],
                        )


    return _split_multiwaits(nc) if split else nc


def _get_nc():
    if "nc" not in _NC_CACHE:
        _NC_CACHE["nc"] = _build_nc()
    return _NC_CACHE["nc"]


def _in_maps(inputs):
    x = np.asarray(inputs["x"], dtype=np.float32)
    ev = np.asarray(inputs["ev"], dtype=np.float32)
    ef = np.asarray(inputs["ef"], dtype=np.float32)
    am = np.asarray(inputs["access_mask"], dtype=np.float32)
    W1 = np.asarray(inputs["W1"], dtype=np.float32)
    b1 = np.asarray(inputs["b1"], dtype=np.float32)
    W2 = np.asarray(inputs["W2"], dtype=np.float32)
    b2 = np.asarray(inputs["b2"], dtype=np.float32)

    # shared weight pack: [128, PK_F] fp32
    pack = np.zeros((128, PK_F), dtype=np.float32)  # cast to bf16 at the end
    pack[:, PK_W1 : PK_W1 + 512] = W1.reshape(2, 128, C).transpose(1, 0, 2).reshape(
        128, 512
    )
    pack[:, PK_W2 : PK_W2 + 512] = W2.reshape(2, 128, C).transpose(1, 0, 2).reshape(
        128, 512
    )
    pack[0, PK_B1 : PK_B1 + C] = b1
    pack[0, PK_B2 : PK_B2 + C] = b2

    maps = []
    for core in range(N_CORES):
        b, qh = core // 2, core % 2
        sl = slice(qh * QSH, (qh + 1) * QSH)
        pk = pack.copy()
        # xT: x[b]^T [C, K] -> [128, 2, 128] (p = c % 128, o = c // 128)
        pk[:, PK_XT : PK_XT + 256] = (
            x[b].T.reshape(2, 128, 128).transpose(1, 0, 2).reshape(128, 256)
        )
        em = np.empty((K, EM_F), dtype=np.float32)
        em[:, EM_EV : EM_EV + 192] = (
            ev[b, sl].transpose(1, 2, 0).reshape(K, 192)
        )
        em[:, EM_MASK : EM_MASK + QSH] = am[b, sl].T
        maps.append(
            {
                "efT": np.ascontiguousarray(
                    ef[b, sl].transpose(1, 0, 2).astype(BF16)
                ),
                "mlp": pk.astype(BF16),
                "evmask": em,
            }
        )
    return maps


def _gather(results):
    out = np.empty((B, Q, D, C), dtype=np.float32)
    for core in range(N_CORES):
        b, qh = core // 2, core % 2
        oraw = results[core]["oraw"].astype(np.float32)  # [4, 3, NG, 512]
        arr = oraw.reshape(4, D, NG, 2, C)  # [s, d, g, f, c]
        out[b, qh * QSH : (qh + 1) * QSH] = (
            arr.transpose(2, 3, 0, 1, 4).reshape(QSH, D, C)
        )
    return out


def _run(inputs, trace=False, **kwargs):
    nc = _get_nc()
    res = run_bass_kernel_spmd(
        nc, _in_maps(inputs), list(range(N_CORES)), trace=trace, **kwargs
    )
    return _gather(res.results), res


def kernel(**inputs) -> np.ndarray:
    out, _ = _run(inputs, trace=False)
    return out
